# revision 1
# baseline (speedup 1.0000x reference)
"""Trainium2 Bass kernel for batched multi-head softmax attention.

Problem: q,k,v [B=4, H=16, N=2048, D=64] fp32.
  out = softmax(q @ k^T / sqrt(D)) @ v   (per b,h)

Sharding: B*H = 64 head-slices, 8 per core across 8 cores. Each core
computes full attention for its 8 heads independently (no collectives).

Per-head algorithm on one core (i = query index, j = key index):
  - Load Q,K natural f32, cast to bf16 (DVE), PE-transpose (2 blocks per
    transpose) into Q^T,K^T [64,2048] bf16.
  - V' = [V | 1] per j-block, bf16: the 65th PV output row accumulates
    sum_j exp = the softmax denominator for free.
  - Flash-style loop: for each i-half icp (1024), for each j-block jb:
      S^T[j,i] = K^T[jb]^T . Q^T    (bf16, d=64 contraction, 2x N=512)
      E = exp(S^T/8)                (one ACT op per jb, PSUM->SBUF bf16)
      O^T += V'[jb]^T . E          (bf16 accumulating matmuls)
  - O^T is PE-transposed back and scaled by 1/denominator (DVE).

Scheduling: QK matmuls emitted with +2 jb lookahead; transposes and
normalization steps are queued as "fillers" and interleaved between the
jb iterations so the PE instruction stream stays dense (the PE HAM clock
gate re-throttles to 1.2 GHz if the PE sees sparse phases).
"""

import numpy as np
from collections import deque

B, H, N, D = 4, 16, 2048, 64
NCORES = 8
HPC = (B * H) // NCORES  # heads per core = 8
NB = N // 128  # 16 j-blocks / i-blocks of 128
SCALE = float(D) ** -0.5

_cache = {}


def _build(hpc=HPC, qk_dt="bfloat16", pv_dt="bfloat16"):
    import concourse.bacc as bacc
    import concourse.tile as tile
    from concourse import mybir
    from concourse.masks import make_identity

    f32 = mybir.dt.float32
    qkd = getattr(mybir.dt, qk_dt)
    pvd = getattr(mybir.dt, pv_dt)
    EXP = mybir.ActivationFunctionType.Exp

    nc = bacc.Bacc("TRN2", target_bir_lowering=False, debug=False)
    q = nc.dram_tensor("q", [hpc, N, D], f32, kind="ExternalInput").ap()
    k = nc.dram_tensor("k", [hpc, N, D], f32, kind="ExternalInput").ap()
    v = nc.dram_tensor("v", [hpc, N, D], f32, kind="ExternalInput").ap()
    out = nc.dram_tensor("out", [hpc, N, D], f32, kind="ExternalOutput").ap()

    with tile.TileContext(nc) as tc:
        with (
            tc.tile_pool(name="consts", bufs=1) as consts,
            tc.tile_pool(name="stage", bufs=2) as stage,
            tc.tile_pool(name="qkt", bufs=2) as qkt,
            tc.tile_pool(name="epool", bufs=3) as epool,
            tc.tile_pool(name="osb", bufs=2) as osb,
            tc.tile_pool(name="outp", bufs=2) as outp,
            tc.tile_pool(name="stp", bufs=2, space="PSUM") as stp,
            tc.tile_pool(name="opsp", bufs=1, space="PSUM") as opsp,
            tc.tile_pool(name="tpp", bufs=2, space="PSUM") as tpp,
        ):
            # Warmup weights: DVE memset (fast launch, no GPSIMD dependency).
            # Always bf16: memset on float32r tiles fails the walrus ISA check.
            warm_w = consts.tile([128, 128], mybir.dt.bfloat16)
            nc.vector.memset(warm_w[:], 0.0)
            # Preload the ACT exp table set (~2.7us) before the first real exp
            # so the PE never stalls on it mid-loop.
            dummy_e = consts.tile([128, 1], f32)
            nc.scalar.activation(dummy_e[:], warm_w[:, 0:1], EXP)

            ident = consts.tile([128, 128], f32)
            make_identity(nc, ident[:])
            identb = consts.tile([128, 128], qkd)
            nc.vector.tensor_copy(identb[:], ident[:])

            # PE warmup: real (non-transpose) matmuls keep the HAM clock gate
            # at K=8/8 (2.4 GHz) through the DMA- and DVE-gated prologue.
            # Transpose-mode ops don't count as PE-busy for HAM. Warm tiles
            # borrow the (still unused) main-loop PSUM slots.
            def warm_burst(n):
                warm = stp.tile([128, 128], f32, tag="st", name="warm")
                for _ in range(n):
                    nc.tensor.matmul(
                        warm[:],
                        warm_w[:, 0:128],
                        warm_w[:, 0:128],
                        start=True,
                        stop=True,
                    )

            fillers = deque()

            def run_fillers(jb, njb=16):
                # spread remaining fillers evenly over the remaining jbs
                left = njb - jb
                k = (len(fillers) + left - 1) // left if left > 0 else len(fillers)
                for _ in range(min(k, len(fillers))):
                    fillers.popleft()()

            def flush_fillers():
                while fillers:
                    fillers.popleft()()

            def emit_loads(h):
                """DMA + casts for head h (SP/DVE only). Returns tiles."""
                q_nat = stage.tile([128, NB * D], f32, tag="q_nat", name="q_nat")
                nc.sync.dma_start(
                    out=q_nat.rearrange("p (b d) -> p b d", b=NB),
                    in_=q[h].rearrange("(b p) d -> p b d", p=128),
                )
                k_nat = stage.tile([128, NB * D], f32, tag="k_nat", name="k_nat")
                nc.sync.dma_start(
                    out=k_nat.rearrange("p (b d) -> p b d", b=NB),
                    in_=k[h].rearrange("(b p) d -> p b d", p=128),
                )
                q_bf = stage.tile([128, NB * D], qkd, tag="q_bf", name="q_bf")
                nc.vector.tensor_copy(q_bf[:], q_nat[:])
                k_bf = stage.tile([128, NB * D], qkd, tag="k_bf", name="k_bf")
                nc.vector.tensor_copy(k_bf[:], k_nat[:])
                v_stage = stage.tile(
                    [128, NB * (D + 1)], f32, tag="v_stage", name="v_stage"
                )
                nc.sync.dma_start(
                    out=v_stage.rearrange("p (b e) -> p b e", b=NB)[:, :, 0:D],
                    in_=v[h].rearrange("(b p) d -> p b d", p=128),
                )
                nc.vector.memset(
                    v_stage.rearrange("p (b e) -> p b e", b=NB)[:, :, D : D + 1], 1.0
                )
                v_r = stage.tile([128, NB * (D + 1)], pvd, tag="v_r", name="v_r")
                nc.vector.tensor_copy(v_r[:], v_stage[:])
                return q_bf, k_bf, v_r

            def queue_transposes(q_bf, k_bf, prologue=False):
                """Build Q^T/K^T [64, 2048] bf16; 2 blocks per PE transpose.

                In the prologue (head 0) the PSUM->SBUF copies alternate
                between DVE and the idle ACT engine and the PSUM tiles
                rotate through 4 slots, keeping the PE transpose stream
                dense enough that the HAM clock gate stays warm."""
                qtr = qkt.tile([64, N], qkd, tag="qt", name="qtr")
                ktr = qkt.tile([64, N], qkd, tag="kt", name="ktr")
                idx = 0
                for src, dst in ((q_bf, qtr), (k_bf, ktr)):
                    for t2 in range(NB // 2):  # 8 paired transposes each
                        idx += 1

                        def tr(src=src, dst=dst, t2=t2, idx=idx):
                            tag = "st" if (prologue and idx % 2) else "tp"
                            tp = tpp.tile([128, 128], qkd, tag=tag, name="tp") \
                                if not (prologue and idx % 2) else \
                                stp.tile([128, 128], qkd, tag="st", name="tp")
                            nc.tensor.matmul(
                                tp[:],
                                src[:, t2 * 2 * D : (t2 * 2 + 2) * D],
                                identb[:, 0:128],
                                is_transpose=True,
                            )
                            t = t2 * 2
                            nc.vector.tensor_copy(
                                dst[:, t * 128 : (t + 1) * 128], tp[0:64, :]
                            )
                            if prologue:
                                nc.scalar.copy(
                                    dst[:, (t + 1) * 128 : (t + 2) * 128],
                                    tp[64:128, :],
                                )
                            else:
                                nc.vector.tensor_copy(
                                    dst[:, (t + 1) * 128 : (t + 2) * 128],
                                    tp[64:128, :],
                                )

                        fillers.append(tr)
                return qtr, ktr

            def queue_norm(o_ps, icp, out_sb):
                """Copy O^T out of PSUM now (frees the accumulators), queue the
                transpose+normalize steps as fillers."""
                o_sbs = []
                for s in range(2):
                    o_sb = osb.tile([65, 512], f32, tag="o_sb", name="o_sb")
                    nc.vector.tensor_copy(o_sb[:], o_ps[s][0:65, :])
                    o_sbs.append(o_sb)
                for s in range(2):
                    for t in range(4):

                        def step(s=s, t=t, icp=icp, out_sb=out_sb, o_sb=o_sbs[s]):
                            pt = tpp.tile([128, 65], f32, tag="tp", name="pt")
                            nc.tensor.matmul(
                                pt[:],
                                o_sb[:, t * 128 : (t + 1) * 128],
                                ident[0:65, 0:65],
                                is_transpose=True,
                            )
                            rec = osb.tile([128, 1], f32, tag="rec", name="rec")
                            nc.vector.reciprocal(rec[:], pt[:, 64:65])
                            blk = icp * 8 + s * 4 + t
                            nc.vector.tensor_scalar_mul(
                                out_sb[:, blk * D : (blk + 1) * D],
                                pt[:, 0:64],
                                rec[:],
                            )

                        fillers.append(step)

            # ---------- prologue: head 0 ----------
            q_bf, k_bf, v_r = emit_loads(0)
            qtr, ktr = queue_transposes(q_bf, k_bf, prologue=True)
            warm_burst(40)  # covers the first DMA+cast latency
            flush_fillers()
            nxt = {}  # head h+1 tiles built during h's icp=1

            for h in range(hpc):
                out_sb = outp.tile([128, NB * D], f32, tag="out_sb", name="out_sb")

                for icp in range(2):
                    if icp == 1 and h + 1 < hpc:
                        # kick off next head's loads; its transposes become
                        # fillers for this icp's loop
                        nq_bf, nk_bf, nv_r = emit_loads(h + 1)
                        nqtr, nktr = queue_transposes(nq_bf, nk_bf)
                        nxt = {"v_r": nv_r, "qtr": nqtr, "ktr": nktr}

                    o_ps0 = opsp.tile([128, 512], f32, tag="o0", name="o_ps0")
                    o_ps1 = opsp.tile([128, 512], f32, tag="o1", name="o_ps1")
                    o_ps = (o_ps0, o_ps1)
                    sts = {}

                    def emit_qk(jb, icp=icp, sts=sts, qtr=qtr, ktr=ktr):
                        st = stp.tile([128, 1024], f32, tag="st", name="st")
                        sts[jb] = st
                        for s in range(2):
                            i0 = icp * 1024 + s * 512
                            nc.tensor.matmul(
                                st[:, s * 512 : (s + 1) * 512],
                                ktr[:, jb * 128 : (jb + 1) * 128],
                                qtr[:, i0 : i0 + 512],
                                start=True,
                                stop=True,
                            )

                    emit_qk(0)
                    emit_qk(1)
                    for jb in range(16):
                        st = sts.pop(jb)
                        er = epool.tile([128, 1024], pvd, tag="e", name="er")
                        nc.scalar.activation(er[:], st[:], EXP, scale=SCALE)
                        if jb + 2 < 16:
                            emit_qk(jb + 2)
                        for s in range(2):
                            nc.tensor.matmul(
                                o_ps[s][0:65, :],
                                v_r[:, jb * 65 : (jb + 1) * 65],
                                er[:, s * 512 : (s + 1) * 512],
                                start=(jb == 0),
                                stop=(jb == 15),
                            )
                        if h == 0 and icp == 0 and jb == 0:
                            # one contiguous burst: trips the HAM SHORT
                            # window so the rest of the run stays at 2.4GHz
                            warm_burst(28)
                        run_fillers(jb)

                    flush_fillers()
                    queue_norm(o_ps, icp, out_sb)

                def out_dma(h=h, out_sb=out_sb):
                    nc.sync.dma_start(
                        out=out[h].rearrange("(b p) d -> p b d", p=128),
                        in_=out_sb.rearrange("p (b d) -> p b d", b=NB),
                    )

                fillers.append(out_dma)
                if nxt:
                    v_r, qtr, ktr = nxt["v_r"], nxt["qtr"], nxt["ktr"]
                    nxt = {}

            flush_fillers()

    nc.compile()
    return nc


def _get_nc():
    if "nc" not in _cache:
        _cache["nc"] = _build()
    return _cache["nc"]


def kernel(q: np.ndarray, k: np.ndarray, v: np.ndarray) -> np.ndarray:
    from concourse.bass_utils import run_bass_kernel_spmd

    nc = _get_nc()
    qf = np.ascontiguousarray(np.asarray(q), dtype=np.float32).reshape(B * H, N, D)
    kf = np.ascontiguousarray(np.asarray(k), dtype=np.float32).reshape(B * H, N, D)
    vf = np.ascontiguousarray(np.asarray(v), dtype=np.float32).reshape(B * H, N, D)
    in_maps = [
        {
            "q": qf[c * HPC : (c + 1) * HPC],
            "k": kf[c * HPC : (c + 1) * HPC],
            "v": vf[c * HPC : (c + 1) * HPC],
        }
        for c in range(NCORES)
    ]
    r = run_bass_kernel_spmd(nc, in_maps, list(range(NCORES)))
    outs = np.concatenate([r.results[c]["out"] for c in range(NCORES)], axis=0)
    return outs.reshape(B, H, N, D).astype(np.float32)

